# revision 7
# baseline (speedup 1.0000x reference)
"""Trainium2 Bass kernel for the Char2RNN problem.

Shapes (hardcoded): B=64, L=512, V=256, E=128, H=512, 8 NeuronCores.

Math (matches the reference):
    diagA[h,e] = A[h,h,e]
    e   = emb[x]                       # [B, L, E]
    xa  = e @ diagA.T                  # [B, L, H]
    xu  = e @ U_w.T + bias             # [B, L, H]
    h_0 = 0
    h_{t+1} = tanh(h_t * xa_t + h_t @ V_w.T + xu_t)
    logits  = hs @ dec_w.T + dec_b     # [B, L, V]
    returns (logits, h_L[None])

Sharding: data-parallel over batch B (8 rows per core); weights replicated.

Device layout (per core, BS = 8 batch rows):
    hsT   [128, (L+1)*32]  bf16   col (t*32 + m*8 + b) = h_t[b, m*128+p]
    xaT   [128, 4*4096]    bf16   col (m*4096 + t*8 + b) = xa[b, t, m*128+p]
    xuT   same layout (includes bias)
    VT    [128, 16*128]    bf16   block (m,k): V_w[m*128:.., k*128:..].T
    eT    [128, 4096]      bf16   col (t*8 + b) = e[b, t, :]
Scan step t: 16 matmuls psum[:, m, :] += VT(m,k).T @ hsT[:, t, k-slice]
(lhsT = V tile is the stationary operand; bf16 enables fast weight load),
then DVE: h*xa, +xu, +psum, then ACT tanh -> hsT slice t+1.
"""

import numpy as np
import ml_dtypes

B, L, V, E, H = 64, 512, 256, 128, 512
NCORES = 8
BS = B // NCORES  # 8 batch rows per core
GB = 32  # columns per timestep in hsT layout: 4 h-blocks x 8 batch

_CACHE = {}


def _build_bass(loop_reps=None):
    import concourse.bass as bass
    import concourse.mybir as mybir
    import concourse.tile as tile
    from concourse import bacc
    from contextlib import ExitStack

    dt = mybir.dt
    bf16 = dt.float16  # "bf16" name kept; actually fp16 for accuracy
    f32 = dt.float32
    AF = mybir.ActivationFunctionType

    nc = bacc.Bacc()

    # ---- I/O ----
    eT_d = nc.dram_tensor("eT", [E, L * BS], bf16, kind="ExternalInput")
    VT_d = nc.dram_tensor("VT", [128, 16 * 128], bf16, kind="ExternalInput")
    AU_d = nc.dram_tensor("AU", [E, 2 * H], bf16, kind="ExternalInput")
    DW_d = nc.dram_tensor("DW", [128, 8 * 128], bf16, kind="ExternalInput")
    BI_d = nc.dram_tensor("BI", [128, 4], f32, kind="ExternalInput")
    DB_d = nc.dram_tensor("DB", [128, 2], f32, kind="ExternalInput")

    lg_d = nc.dram_tensor("logits_sh", [BS * L, V], f32, kind="ExternalOutput")
    ht_d = nc.dram_tensor("hT_sh", [128, GB], f32, kind="ExternalOutput")

    with tile.TileContext(nc) as tc, ExitStack() as ctx:
        const = ctx.enter_context(tc.tile_pool(name="const", bufs=1))
        big = ctx.enter_context(tc.tile_pool(name="big", bufs=1))
        work = ctx.enter_context(tc.tile_pool(name="work", bufs=3))
        outp = ctx.enter_context(tc.tile_pool(name="outp", bufs=4))
        ps_pre = ctx.enter_context(
            tc.tile_pool(name="ps_pre", bufs=3, space=bass.MemorySpace.PSUM)
        )
        ps_scan = ctx.enter_context(
            tc.tile_pool(name="ps_scan", bufs=2, space=bass.MemorySpace.PSUM)
        )
        ps_dec = ctx.enter_context(
            tc.tile_pool(name="ps_dec", bufs=3, space=bass.MemorySpace.PSUM)
        )

        def body():
            # ---- load inputs ----
            eT = big.tile([128, L * BS], bf16)
            nc.sync.dma_start(eT[:], eT_d[:])
            VT = const.tile([128, 16 * 128], bf16)
            nc.sync.dma_start(VT[:], VT_d[:])
            AU = const.tile([128, 2 * H], bf16)
            nc.sync.dma_start(AU[:], AU_d[:])
            DW = const.tile([128, 8 * 128], bf16)
            nc.sync.dma_start(DW[:], DW_d[:])
            BI = const.tile([128, 4], f32)
            nc.sync.dma_start(BI[:], BI_d[:])
            DB = const.tile([128, 2], f32)
            nc.sync.dma_start(DB[:], DB_d[:])

            xaT = big.tile([128, 4 * 4096], bf16)
            xuT = big.tile([128, 4 * 4096], bf16)
            hsT = big.tile([128, (L + 1) * GB], bf16)
            nc.vector.memset(hsT[:, 0:GB], 0.0)

            # ---- precompute xa / xu for all timesteps ----
            # psum[pm, c] = sum_e AU[e, src*512 + m*128 + pm] * eT[e, c]
            NCH = (L * BS) // 512  # 8 chunks of 512 columns
            for src, dst in ((0, xaT), (1, xuT)):
                for m in range(4):
                    for ch in range(NCH):
                        pp = ps_pre.tile([128, 512], f32, tag="pp")
                        nc.tensor.matmul(
                            pp[:],
                            AU[:, src * H + m * 128 : src * H + (m + 1) * 128],
                            eT[:, ch * 512 : (ch + 1) * 512],
                            start=True,
                            stop=True,
                        )
                        dcol = m * 4096 + ch * 512
                        if src == 1:
                            # xu gets the bias folded in (per-partition within block m)
                            nc.scalar.activation(
                                dst[:, dcol : dcol + 512],
                                pp[:],
                                AF.Identity,
                                bias=BI[:, m : m + 1],
                            )
                        else:
                            nc.vector.tensor_copy(dst[:, dcol : dcol + 512], pp[:])

            # rearranged views for per-step slicing
            xa_r = xaT.rearrange("p (m c) -> p m c", m=4)  # [128, 4, 4096]
            xu_r = xuT.rearrange("p (m c) -> p m c", m=4)
            hs_r = hsT.rearrange("p (t g) -> p t g", g=GB)  # [128, 513, 32]

            # ---- sequential scan ----
            for t in range(L):
                hp = hs_r[:, t, :].rearrange("p (m b) -> p m b", m=4)  # [128,4,8]
                tmp = work.tile([128, 4, BS], f32, tag="tmp")
                nc.vector.tensor_mul(tmp[:], hp, xa_r[:, :, t * BS : (t + 1) * BS])
                tmp2 = work.tile([128, 4, BS], f32, tag="tmp2")
                nc.vector.tensor_add(tmp2[:], tmp[:], xu_r[:, :, t * BS : (t + 1) * BS])

                ps = ps_scan.tile([128, 4, BS], f32, tag="ps")
                for m in range(4):
                    for k in range(4):
                        nc.tensor.matmul(
                            ps[:, m, :],
                            VT[:, (m * 4 + k) * 128 : (m * 4 + k + 1) * 128],
                            hs_r[:, t, k * BS : (k + 1) * BS],
                            start=(k == 0),
                            stop=(k == 3),
                        )
                t4 = work.tile([128, 4, BS], f32, tag="t4")
                nc.vector.tensor_add(t4[:], tmp2[:], ps[:])
                nc.scalar.activation(
                    hs_r[:, t + 1, :].rearrange("p (m b) -> p m b", m=4),
                    t4[:],
                    AF.Tanh,
                )

            # ---- decoder ----
            # out[pv, t', b] = sum_k sum_pk DW[pk, (mv*4+k)*128+pv] * h_{t}[b, k*128+pk]
            lg_v = lg_d.rearrange("(b t) v -> v t b", b=BS)  # [256, 512, 8]
            hs_dec = hsT[:, GB:].rearrange("p (t g) -> p t g", g=GB)  # [128, 512, 32]
            TCH = 64  # timesteps per decoder chunk -> N = 64*8 = 512
            for mv in range(2):
                for ch in range(L // TCH):
                    pd = ps_dec.tile([128, TCH, BS], f32, tag="pd")
                    for k in range(4):
                        nc.tensor.matmul(
                            pd[:],
                            DW[:, (mv * 4 + k) * 128 : (mv * 4 + k + 1) * 128],
                            hs_dec[:, ch * TCH : (ch + 1) * TCH, k * BS : (k + 1) * BS],
                            start=(k == 0),
                            stop=(k == 3),
                        )
                    lg = outp.tile([128, TCH, BS], f32, tag="lg")
                    nc.scalar.activation(
                        lg[:], pd[:], AF.Identity, bias=DB[:, mv : mv + 1]
                    )
                    for b in range(BS):
                        nc.sync.dma_start(
                            lg_v[mv * 128 : (mv + 1) * 128, ch * TCH : (ch + 1) * TCH, b],
                            lg[:, :, b],
                        )

            # ---- final hidden state ----
            ht = outp.tile([128, GB], f32, tag="ht")
            nc.vector.tensor_copy(ht[:], hs_r[:, L, :])
            nc.sync.dma_start(ht_d[:], ht[:])

        if loop_reps is None:
            body()
        else:
            with tc.For_i(0, loop_reps, 1):
                body()

    nc.compile()
    return nc


def _prep_inputs(x, emb, A, U_w, V_w, bias, dec_w, dec_b):
    """Host-side marshaling into the device layouts (bf16)."""
    bf = np.float16
    x = np.asarray(x)
    emb = np.asarray(emb, dtype=np.float32)
    A = np.asarray(A, dtype=np.float32)
    U_w = np.asarray(U_w, dtype=np.float32)
    V_w = np.asarray(V_w, dtype=np.float32)
    bias = np.asarray(bias, dtype=np.float32)
    dec_w = np.asarray(dec_w, dtype=np.float32)
    dec_b = np.asarray(dec_b, dtype=np.float32)

    diagA = A[np.arange(H), np.arange(H), :]  # [H, E]

    AU = np.concatenate([diagA.T, U_w.T], axis=1).astype(bf)  # [128, 1024]
    VT = np.concatenate(
        [
            V_w[m * 128 : (m + 1) * 128, k * 128 : (k + 1) * 128].T
            for m in range(4)
            for k in range(4)
        ],
        axis=1,
    ).astype(bf)  # [128, 2048]
    DW = np.concatenate(
        [
            dec_w[mv * 128 : (mv + 1) * 128, k * 128 : (k + 1) * 128].T
            for mv in range(2)
            for k in range(4)
        ],
        axis=1,
    ).astype(bf)  # [128, 1024]
    BI = bias.reshape(4, 128).T.astype(np.float32).copy()  # [128, 4]
    DB = dec_b.reshape(2, 128).T.astype(np.float32).copy()  # [128, 2]

    e_full = emb[x]  # [B, L, E] f32

    in_maps = []
    for c in range(NCORES):
        esh = e_full[c * BS : (c + 1) * BS]  # [BS, L, E]
        eT = np.ascontiguousarray(esh.transpose(2, 1, 0).reshape(E, L * BS)).astype(bf)
        in_maps.append(
            {"eT": eT, "VT": VT, "AU": AU, "DW": DW, "BI": BI, "DB": DB}
        )
    return in_maps


def _run(inputs, trace=False):
    from concourse.bass_utils import run_bass_kernel_spmd

    if "nc" not in _CACHE:
        _CACHE["nc"] = _build_bass()
    nc = _CACHE["nc"]

    in_maps = _prep_inputs(**inputs)
    res = run_bass_kernel_spmd(
        nc, in_maps, core_ids=list(range(NCORES)), trace=trace
    )

    logits = np.empty((B, L, V), dtype=np.float32)
    hT = np.empty((B, H), dtype=np.float32)
    for c, r in enumerate(res.results):
        logits[c * BS : (c + 1) * BS] = r["logits_sh"].reshape(BS, L, V)
        # ht buf [128, 32]: col m*8+b holds h_L[b, m*128+p]
        hbuf = r["hT_sh"].reshape(128, 4, BS)
        hT[c * BS : (c + 1) * BS] = hbuf.transpose(2, 1, 0).reshape(BS, H)
    return (logits, hT[None]), res


def kernel(**inputs):
    out, _ = _run(inputs, trace=False)
    return out

